# revision 7
# baseline (speedup 1.0000x reference)
"""Trainium2 Bass kernel for nn_DecompGrid (factorized-grid embedding lookup).

Computation (per point, C=16 channels):
    out[n, 0:16]  = trilerp(grid3d, xyz) * bilerp(p0, (c1,c2)) * bilerp(p1, (c0,c2)) * bilerp(p2, (c0,c1))
    out[n, 16:32] = linelerp(line0, x[:, 3])

The SWDGE descriptor generation on the Pool engine (~2.8 ns/descriptor,
serialized) is the bottleneck, so only grid3d and plane1 are fetched via
dma_gather (2 descriptors/point instead of 4).  plane0 and plane2 are
evaluated on the otherwise-idle PE: points are sorted into (gy, gx-pair)
windows; within a window plane0 touches <= 4x18 positions and plane2
<= 4x6, so a per-window stationary [96, 32] (p0 patch ++ p2 patch ->
16+16 output channels) times a host-built sparse-weight moving tile
computes both bilerps for 32-point blocks.  The PSUM result (column
layout) returns to the row layout via an HWDGE dma transpose, and the
final product is a pair of cheap row-layout DVE multiplies.

  - Host: compute cell indices + per-corner lerp weight products, route
    points to the 8 cores by grid z-slab, sort by window, pad windows to
    32-point blocks, shuffle blocks (spreads gather addresses; avoids the
    HBM locality cliff of fully sorted gathers), and build fp16 dup-block
    tables whose rows hold a full interpolation neighborhood:
      grid:   (16ch, 8 corners) fp16 = 256B per row, 8*64*64 rows per slab
      plane1: (16ch, 4 corners) fp16 = 128B + 128B pad, 128*128 rows
  - Device (per chunk of 128*S points): fp16 corner weights + wrapped
    int16 indices, 2x2 SWDGE dma_gather, per-block PE matmuls for p0/p2,
    fp16 DVE weighted combine, store (128, S, 16) fp16 (host upcasts).
  - Line lerp on PE: stationary (64, 16) line table, moving host-built
    (64, CHUNK) sparse weight columns, PSUM -> fp16 via the scalar engine.
"""

import math
import numpy as np

import concourse.bacc as bacc
import concourse.bass as bass
import concourse.tile as tile
from concourse import mybir
from concourse import bass_utils

# ---------------- problem constants (hardcoded) ----------------
N = 1_000_000
C = 16
D = H = W = 128        # grid3d spatial dims
HP = WP = 256          # plane dims
LL = 64                # line length
NCORES = 8

S = 32                 # point-groups per partition per chunk
CHUNK = 128 * S        # points per chunk
NW = 12                # weight columns per point (8 grid + 4 plane1)
BLK = 32               # points per p0/p2 matmul block
NBLK = CHUNK // BLK    # 128 blocks per chunk
KP0 = 72               # plane0 stationary rows: 4 py * 18 pz
KP2 = 24               # plane2 stationary rows: 4 py * 6 px
KST = KP0 + KP2        # 96
NWIN = 64 * 32         # (gy, gx-pair) windows per core

F32 = mybir.dt.float32
F16 = mybir.dt.float16
I16 = mybir.dt.int16


# ---------------- walrus / tile workarounds ----------------
_PATCHED = False


def _apply_patches():
    """This container's walrus rejects >1 sync-wait command on the Tile tail
    drain; split the waits into explicit wait_ge instructions."""
    global _PATCHED
    if _PATCHED:
        return
    _PATCHED = True
    import concourse.tile as tile_mod
    from concourse.tile import ScopedClock

    def _drain_and_barrier_split(self, tick_clock, wait_clock):
        drain_inst = self.nc.sync.drain()
        wait_clock.add_sem_waits(
            drain_inst.ins, ScopedClock({None: tick_clock.global_clock})
        )
        si = drain_inst.ins.sync_info
        if si is not None and len(si.on_wait) > 1:
            assert self.sems is not None
            by_name = {h.name: h for h in self.sems.allocated().values()}
            keep, spill = [], []
            for w in si.on_wait:
                h = by_name.get(w.ant_name)
                if h is None or len(keep) < 1:
                    keep.append(w)
                else:
                    spill.append((h, w.wait_value))
            si.on_wait = keep
            for h, v in spill:
                self.nc.sync.wait_ge(h, v)
        self.nc.all_engine_barrier()
        assert self.sems is not None
        popped = self.nc._tile_sem_poison_stack.pop()
        assert popped is self._sem_poison
        self.nc.clear_and_free_semaphores(list(self.sems.allocated().values()))
        self.nc.all_engine_barrier()

    tile_mod.TileContext._drain_and_barrier = _drain_and_barrier_split


# ---------------- device program ----------------

def build_program(nchunks: int):
    """Build + compile the SPMD bass program for `nchunks` chunks per core."""
    _apply_patches()
    nc = bacc.Bacc(
        "TRN2",
        num_devices=1,
        debug=False,
        target_bir_lowering=False,
        num_swdge_queues=4,
    )
    FS = S * 8   # idx cols per partition

    wts_d = nc.dram_tensor("wts", (nchunks, 128, S * NW), F16, kind="ExternalInput").ap()
    ig_d = nc.dram_tensor("idxg", (nchunks, 128, FS), I16, kind="ExternalInput").ap()
    ip1_d = nc.dram_tensor("idxp1", (nchunks, 128, FS), I16, kind="ExternalInput").ap()
    al_d = nc.dram_tensor("al", (nchunks, 64, CHUNK), F16, kind="ExternalInput").ap()
    w0_d = nc.dram_tensor("w0", (nchunks, KST, CHUNK), F16, kind="ExternalInput").ap()
    st_d = nc.dram_tensor("st", (nchunks, KST, NBLK * BLK), F16, kind="ExternalInput").ap()
    gtab = nc.dram_tensor("gtab", (8 * 64 * 64, 128), F16, kind="ExternalInput").ap()
    p1tab = nc.dram_tensor("p1tab", (128 * 128, 128), F16, kind="ExternalInput").ap()
    ltab = nc.dram_tensor("ltab", (LL, C), F16, kind="ExternalInput").ap()
    out_d = nc.dram_tensor("out", (nchunks, 128, S * 16), F16, kind="ExternalOutput").ap()
    oln_d = nc.dram_tensor("oln", (nchunks, 16, CHUNK), F16, kind="ExternalOutput").ap()

    mul = mybir.AluOpType.mult
    add = mybir.AluOpType.add

    with tile.TileContext(nc) as tc:
        with tc.tile_pool(name="pconst", bufs=1) as pconst, \
             tc.tile_pool(name="pin", bufs=3) as pin, \
             tc.tile_pool(name="pval", bufs=4) as pval, \
             tc.tile_pool(name="ptmp", bufs=2) as ptmp, \
             tc.tile_pool(name="pps", bufs=4, space="PSUM") as pps, \
             tc.tile_pool(name="pps2", bufs=4, space="PSUM") as pps2, \
             tc.tile_pool(name="pout", bufs=2) as pout:
            lsb = pconst.tile([LL, C], F16, tag="lsb")
            nc.sync.dma_start(out=lsb[:], in_=ltab)

            for k in range(nchunks):
                # ---- loads ----
                wts = pin.tile([128, S, NW], F16, tag="wts")
                nc.sync.dma_start(out=wts[:], in_=wts_d[k].rearrange("p (s q) -> p s q", q=NW))
                ig = pin.tile([128, FS], I16, tag="ig")
                nc.sync.dma_start(out=ig[:], in_=ig_d[k])
                ip1 = pin.tile([128, FS], I16, tag="ip1")
                nc.sync.dma_start(out=ip1[:], in_=ip1_d[k])
                al = pin.tile([64, CHUNK], F16, tag="al")
                nc.sync.dma_start(out=al[:], in_=al_d[k])
                w0t = pin.tile([KST, CHUNK], F16, tag="w0t")
                nc.sync.dma_start(out=w0t[:], in_=w0_d[k])
                stt = pin.tile([KST, NBLK * BLK], F16, tag="stt")
                nc.sync.dma_start(out=stt[:], in_=st_d[k])

                # ---- gathers (grid + plane1; one table per SWDGE queue,
                # each split into two half-chunk gathers issued round-robin
                # so the queues stay fed across chunk boundaries) ----
                vg = pval.tile([128, S, 128], F16, tag="vg")
                vp1 = pval.tile([128, S, 128], F16, tag="vp1")
                tabs = ((vg, gtab, ig), (vp1, p1tab, ip1))
                HS, HC, HF = S // 2, CHUNK // 2, FS // 2
                for h in range(2):
                    for v, tab, ip in tabs:
                        nc.gpsimd.dma_gather(
                            v[:, HS * h:HS * (h + 1), :], tab,
                            ip[:, HF * h:HF * (h + 1)], HC, HC, 128,
                            queue_num=0, single_packet=False)

                # ---- line lerp on PE: (64,16)^T @ (64,512) x 8 ----
                oln = pout.tile([16, CHUNK], F16, tag="oln")
                for j in range(CHUNK // 512):
                    ps = pps.tile([16, 512], F32, tag="ps")
                    nc.tensor.matmul(ps[:], lhsT=lsb[:], rhs=al[:, 512 * j:512 * (j + 1)])
                    nc.scalar.copy(out=oln[:, 512 * j:512 * (j + 1)], in_=ps[:])
                nc.sync.dma_start(out=oln_d[k], in_=oln[:])

                # ---- plane0 + plane2 on PE: per-block [96,32] stationary ----
                p02sb = ptmp.tile([32, CHUNK], F16, tag="p02sb")
                for g in range(CHUNK // 512):
                    ps2 = pps2.tile([32, 512], F32, tag="ps2")
                    for b in range(512 // BLK):
                        blk = g * (512 // BLK) + b
                        nc.tensor.matmul(
                            ps2[:, BLK * b:BLK * (b + 1)],
                            lhsT=stt[:, BLK * blk:BLK * (blk + 1)],
                            rhs=w0t[:, BLK * blk:BLK * (blk + 1)],
                            start=True, stop=True)
                    nc.scalar.copy(out=p02sb[:, 512 * g:512 * (g + 1)], in_=ps2[:])
                p02row = ptmp.tile([128, S, 32], F16, tag="p02row")
                nc.sync.dma_start_transpose(out=p02row[:], in_=p02sb[:])

                out_t = pout.tile([128, S, 16], F16, tag="out")
                out16 = out_t[:, :, 0:16]

                # ---- combine: grid (row = 16ch x 8 corners, fp16 2x mode) ----
                vg4 = vg[:].rearrange("p s (c k) -> p s c k", k=8)
                nc.vector.tensor_tensor(
                    out=vg4, in0=vg4,
                    in1=wts[:, :, 0:8].unsqueeze(2).broadcast_to([128, S, 16, 8]),
                    op=mul,
                )
                nc.vector.tensor_tensor(
                    out=vg4[:, :, :, 0:4], in0=vg4[:, :, :, 0:4], in1=vg4[:, :, :, 4:8], op=add)
                nc.vector.tensor_tensor(
                    out=vg4[:, :, :, 0:2], in0=vg4[:, :, :, 0:2], in1=vg4[:, :, :, 2:4], op=add)
                nc.vector.tensor_tensor(
                    out=out16, in0=vg4[:, :, :, 0], in1=vg4[:, :, :, 1], op=add)

                # ---- combine: plane1 (row = 16ch x 4 corners + pad) ----
                v4 = vp1[:, :, 0:64].rearrange("p s (c k) -> p s c k", k=4)
                nc.vector.tensor_tensor(
                    out=v4, in0=v4,
                    in1=wts[:, :, 8:12].unsqueeze(2).broadcast_to([128, S, 16, 4]),
                    op=mul,
                )
                nc.vector.tensor_tensor(
                    out=v4[:, :, :, 0:2], in0=v4[:, :, :, 0:2], in1=v4[:, :, :, 2:4],
                    op=add)
                tsum = ptmp.tile([128, S, 16], F16, tag="ts1")
                nc.vector.tensor_tensor(
                    out=tsum[:], in0=v4[:, :, :, 0], in1=v4[:, :, :, 1], op=add)
                nc.vector.tensor_tensor(
                    out=out16, in0=out16, in1=tsum[:], op=mul)

                # ---- multiply in plane0 / plane2 (row layout) ----
                nc.vector.tensor_tensor(
                    out=out16, in0=out16, in1=p02row[:, :, 0:16], op=mul)
                nc.vector.tensor_tensor(
                    out=out16, in0=out16, in1=p02row[:, :, 16:32], op=mul)

                # ---- store ----
                nc.sync.dma_start(out=out_d[k], in_=out_t[:].rearrange("p s q -> p (s q)"))

    # Spread gathers across the 4 SWDGE queues.  Tile assigned each Pool-DMA
    # a DMASW{lane} sem in scheduled order; a sem must always be fed by the
    # same queue, so derive queue_num = lane % 4.
    for bb in nc.m.functions[0].blocks:
        for inst in bb.instructions:
            if isinstance(inst, mybir.InstDMAGatherAnt):
                si = inst.sync_info
                for u in (si.on_update if si else []):
                    if u.ant_name.startswith("DMASW"):
                        lane = int(u.ant_name[5:].split("_")[0])
                        inst.queue_num = lane % 4
                        break
    nc.compile()
    return nc


_PROGRAM_CACHE = {}


def _get_program(nchunks: int):
    if nchunks not in _PROGRAM_CACHE:
        _PROGRAM_CACHE[nchunks] = build_program(nchunks)
    return _PROGRAM_CACHE[nchunks]


# ---------------- host-side preparation ----------------

def _split_idx_host(p, lo, hi):
    """Clamped floor + weight, matching the reference within [lo, hi+1]."""
    i0 = np.clip(np.floor(p), lo, hi).astype(np.int32)
    w = (p - i0.astype(np.float32)).astype(np.float32)
    return i0, w


def _build_tables(grid3d, plane1):
    gT = np.ascontiguousarray(grid3d.transpose(1, 2, 3, 0)).astype(np.float16)  # (D,H,W,C)
    # per-core z-slab dup-block tables: core c owns z-origins 63+8c .. 63+8c+7
    # row layout: (16 ch, 8 corners) with corner = 4dz+2dy+dx
    gtabs = []
    for c in range(NCORES):
        z0 = 63 + 8 * c
        blk = np.empty((8, 64, 64, C, 2, 2, 2), np.float16)
        for dz in range(2):
            for dy in range(2):
                for dx in range(2):
                    blk[:, :, :, :, dz, dy, dx] = gT[
                        z0 + dz:z0 + dz + 8, 63 + dy:127 + dy, 63 + dx:127 + dx, :]
        gtabs.append(blk.reshape(8 * 64 * 64, 128))

    # plane1 rows: elems 0:64 = (16 ch, 4 corners) with corner = 2dy+dx
    pT = np.ascontiguousarray(plane1.transpose(1, 2, 0)).astype(np.float16)  # (H,W,C)
    blk = np.zeros((128, 128, 128), np.float16)
    core = blk[:, :, 0:64].reshape(128, 128, C, 2, 2)
    for dy in range(2):
        for dx in range(2):
            core[:, :, :, dy, dx] = pT[127 + dy:255 + dy, 127 + dx:255 + dx, :]
    p1t = blk.reshape(128 * 128, 128)
    return gtabs, p1t


def _wrap_idx(idx_slot, nchunks):
    """(cap,) int in slot order -> (nchunks, 128, 8S) int16 wrapped layout."""
    a = idx_slot.astype(np.int16).reshape(nchunks, S, 8, 16)
    a = a.transpose(0, 3, 1, 2).reshape(nchunks, 16, 8 * S)
    return np.ascontiguousarray(np.tile(a, (1, 8, 1)))


def _corner_weights(wgh, w1y, w1x):
    """(npts, NW) f16: grid 8 corner products + plane1 4 corner products."""
    npts = wgh.shape[0]
    wts = np.zeros((npts, NW), np.float16)
    one = np.float32(1.0)
    wx, wy, wz = wgh[:, 0], wgh[:, 1], wgh[:, 2]
    zs = (one - wz, wz)
    ys = (one - wy, wy)
    xs = (one - wx, wx)
    for dz in range(2):
        for dy in range(2):
            zy = zs[dz] * ys[dy]
            for dx in range(2):
                wts[:, 4 * dz + 2 * dy + dx] = zy * xs[dx]
    # plane1 ((c0,c2) -> W from x, H from z): corner = 2*dH + dW
    cys = (one - w1y, w1y)
    cxs = (one - w1x, w1x)
    for dy in range(2):
        for dx in range(2):
            wts[:, 8 + 2 * dy + dx] = cys[dy] * cxs[dx]
    return wts


def kernel(x, grid3d, plane0, plane1, plane2, line0):
    x = np.asarray(x, np.float32)
    grid3d = np.asarray(grid3d, np.float32)
    plane0 = np.asarray(plane0, np.float32)
    plane1 = np.asarray(plane1, np.float32)
    plane2 = np.asarray(plane2, np.float32)
    line0 = np.asarray(line0, np.float32)

    npts_total = x.shape[0]
    half = np.float32(0.5)
    one = np.float32(1.0)

    # coordinates in the reference's f32 arithmetic order
    pg = ((x[:, 0:3] + one) * half) * np.float32(D - 1)   # grid:  coords 0,1,2
    pp = ((x[:, 0:3] + one) * half) * np.float32(HP - 1)  # plane coords
    pl = x[:, 3] * np.float32(LL - 1)

    i0g, wgh = _split_idx_host(pg, 63, 126)
    i0p, wph = _split_idx_host(pp, 127, 254)
    i0l, wlh = _split_idx_host(pl, 0, 62)

    # z-slab routing (grid z = coord 2)
    slab = (i0g[:, 2] - 63) >> 3

    # window key within core: (gy_local, gx_pair)
    gyl = i0g[:, 1] - 63          # 0..63
    gxp = (i0g[:, 0] - 63) >> 1   # 0..31
    wkey = gyl * 32 + gxp         # 0..2047

    # per-point table indices (slab-local grid)
    idx_g = ((i0g[:, 2] - 63 - 8 * slab) * 64 + (i0g[:, 1] - 63)) * 64 + (i0g[:, 0] - 63)
    idx_p1 = (i0p[:, 2] - 127) * 128 + (i0p[:, 0] - 127)

    # plane1 weights: H from z (coord 2), W from x (coord 0)
    wts = _corner_weights(wgh, wph[:, 2], wph[:, 0])
    wl16 = wlh.astype(np.float16)
    wl16c = (np.float32(1.0) - wlh).astype(np.float16)

    gtabs, p1t = _build_tables(grid3d, plane1)
    ltab = np.ascontiguousarray(line0.T).astype(np.float16)  # (L, C)

    # pad planes for stationary-patch builds (indices clipped below anyway)
    p0pad = plane0.astype(np.float16)
    p2pad = plane2.astype(np.float16)

    rng = np.random.default_rng(1234)

    # ---- per-core window packing (two passes: sizes, then streams) ----
    core_data = []
    max_padded = 0
    for c in range(NCORES):
        selc = np.nonzero(slab == c)[0]
        wk = wkey[selc]
        order = np.argsort(wk, kind="stable")
        selc = selc[order]
        wk = wk[order]
        counts = np.bincount(wk, minlength=NWIN)
        padded = ((counts + BLK - 1) // BLK) * BLK
        offs = np.zeros(NWIN + 1, np.int64)
        offs[1:] = np.cumsum(padded)
        total = int(offs[-1])
        max_padded = max(max_padded, total)
        core_data.append((selc, wk, counts, padded, offs))

    nchunks = max(1, math.ceil(max_padded / CHUNK))
    cap = nchunks * CHUNK
    nblk_cap = cap // BLK

    in_maps = []
    sel_blocked_all = []
    for c in range(NCORES):
        selc, wk, counts, padded, offs = core_data[c]
        total = int(offs[-1])

        # window-first-point fill, then scatter real points to their slots
        first_of_win = np.zeros(NWIN, np.int64)
        nz = counts > 0
        # first occurrence index in selc for each nonempty window
        starts = np.zeros(NWIN + 1, np.int64)
        starts[1:] = np.cumsum(counts)
        first_of_win[nz] = selc[starts[:-1][nz]]

        sel_blocked = np.repeat(first_of_win, padded)
        win_of_slot = np.repeat(np.arange(NWIN), padded)
        # rank of each sorted point within its window
        rank = np.arange(selc.shape[0], dtype=np.int64) - np.repeat(starts[:-1], counts)
        slot = offs[:-1][wk] + rank
        sel_blocked[slot] = selc
        # tail fill: replicate the first block
        if cap > total:
            pad_n = cap - total
            if total >= BLK:
                sel_blocked = np.concatenate(
                    [sel_blocked, np.tile(sel_blocked[:BLK], pad_n // BLK)])
                win_of_slot = np.concatenate(
                    [win_of_slot, np.tile(win_of_slot[:BLK], pad_n // BLK)])
            else:
                sel_blocked = np.concatenate(
                    [sel_blocked, np.repeat(sel_blocked[:1] if total else [0], pad_n)])
                win_of_slot = np.concatenate(
                    [win_of_slot, np.repeat(win_of_slot[:1] if total else [0], pad_n)])

        # shuffle blocks to spread gather addresses
        perm = rng.permutation(nblk_cap)
        sel_blocked = sel_blocked.reshape(nblk_cap, BLK)[perm].reshape(cap)
        win_of_slot = win_of_slot.reshape(nblk_cap, BLK)[perm].reshape(cap)
        win_of_block = win_of_slot.reshape(nblk_cap, BLK)[:, 0]

        # ---- window bases (empirical minima; exact, no ulp hazards) ----
        pz_pt = i0p[selc, 2]
        py_pt = i0p[selc, 1]
        px_pt = i0p[selc, 0]
        big = np.full(NWIN, 32767, np.int32)
        pzb = big.copy(); pyb = big.copy(); pxb = big.copy()
        np.minimum.at(pzb, wk, pz_pt)
        np.minimum.at(pyb, wk, py_pt)
        np.minimum.at(pxb, wk, px_pt)
        # verify spans fit the stationary patches
        mx = np.zeros(NWIN, np.int32)
        np.maximum.at(mx, wk, pz_pt)
        assert ((mx - pzb)[nz] <= 16).all(), "pz span too large"
        mx[:] = 0
        np.maximum.at(mx, wk, py_pt)
        assert ((mx - pyb)[nz] <= 2).all(), "py span too large"
        mx[:] = 0
        np.maximum.at(mx, wk, px_pt)
        assert ((mx - pxb)[nz] <= 4).all(), "px span too large"
        pzb[~nz] = 127; pyb[~nz] = 127; pxb[~nz] = 127

        # ---- per-window stationaries [KST, 32] ----
        stat_win = np.zeros((NWIN, KST, 32), np.float16)
        for dz in range(18):
            zi = np.minimum(pzb + dz, 255)
            for dy in range(4):
                yi = np.minimum(pyb + dy, 255)
                stat_win[:, dz * 4 + dy, 0:16] = p0pad[:, zi, yi].T
        for dy in range(4):
            yi = np.minimum(pyb + dy, 255)
            for dx in range(6):
                xi = np.minimum(pxb + dx, 255)
                stat_win[:, KP0 + dy * 6 + dx, 16:32] = p2pad[:, yi, xi].T
        stat_blocks = stat_win[win_of_block]                     # (nblk_cap, KST, 32)
        st = stat_blocks.reshape(nchunks, NBLK, KST, BLK)
        st = np.ascontiguousarray(st.transpose(0, 2, 1, 3).reshape(nchunks, KST, NBLK * BLK))

        # ---- sparse weight columns W0 [cap, KST] ----
        pidx = sel_blocked
        wlot = win_of_slot
        z0o = (i0p[pidx, 2] - pzb[wlot]).astype(np.int64)
        y0o = (i0p[pidx, 1] - pyb[wlot]).astype(np.int64)
        x0o = (i0p[pidx, 0] - pxb[wlot]).astype(np.int64)
        wz1 = wph[pidx, 2]; wy1 = wph[pidx, 1]; wx1 = wph[pidx, 0]
        one = np.float32(1.0)
        W0 = np.zeros((cap, KST), np.float16)
        ar = np.arange(cap)
        for dz in range(2):
            vz = (one - wz1) if dz == 0 else wz1
            for dy in range(2):
                vy = (one - wy1) if dy == 0 else wy1
                W0[ar, (z0o + dz) * 4 + (y0o + dy)] = vz * vy
        for dy in range(2):
            vy = (one - wy1) if dy == 0 else wy1
            for dx in range(2):
                vx = (one - wx1) if dx == 0 else wx1
                W0[ar, KP0 + (y0o + dy) * 6 + (x0o + dx)] = vy * vx
        w0 = np.ascontiguousarray(
            W0.reshape(nchunks, CHUNK, KST).transpose(0, 2, 1))

        # ---- gather weights / indices (slot i == blocked position i:
        # both the gather and the dma transpose map position i to
        # (partition i%128, group i//128)) ----
        wtsc = wts[pidx].reshape(nchunks, S, 128, NW).transpose(0, 2, 1, 3)
        wtsc = np.ascontiguousarray(wtsc.reshape(nchunks, 128, S * NW))
        idxg_slot = idx_g[pidx]
        idxp1_slot = idx_p1[pidx]

        # ---- line lerp columns (blocked-sorted order) ----
        alc = np.zeros((cap, LL), np.float16)
        i0s = i0l[pidx]
        alc[ar, i0s] = wl16c[pidx]
        alc[ar, i0s + 1] = wl16[pidx]
        alc = np.ascontiguousarray(alc.reshape(nchunks, CHUNK, LL).transpose(0, 2, 1))

        in_maps.append({
            "wts": wtsc,
            "idxg": _wrap_idx(idxg_slot, nchunks),
            "idxp1": _wrap_idx(idxp1_slot, nchunks),
            "al": alc,
            "w0": w0,
            "st": st,
            "gtab": gtabs[c],
            "p1tab": p1t,
            "ltab": ltab,
        })
        sel_blocked_all.append(sel_blocked)

    nc = _get_program(nchunks)
    res = bass_utils.run_bass_kernel_spmd(nc, in_maps, core_ids=list(range(NCORES)))
    kernel.last_results = res

    out = np.empty((npts_total, 32), np.float32)
    for c in range(NCORES):
        sel_blocked = sel_blocked_all[c]
        # out tile (k, p, s, 16): slot (p, s) holds blocked position s*128+p
        o = res.results[c]["out"].astype(np.float32).reshape(nchunks, 128, S, 16)
        o = o.transpose(0, 2, 1, 3).reshape(cap, 16)
        out[sel_blocked, 0:16] = o
        ol = res.results[c]["oln"].astype(np.float32)  # (nchunks, 16, CHUNK)
        ol = ol.transpose(0, 2, 1).reshape(cap, 16)
        out[sel_blocked, 16:32] = ol
    return out


# revision 8
# speedup vs baseline: 1.0863x; 1.0863x over previous
"""Trainium2 Bass kernel for nn_DecompGrid (factorized-grid embedding lookup).

Computation (per point, C=16 channels):
    out[n, 0:16]  = trilerp(grid3d, xyz) * bilerp(p0, (c1,c2)) * bilerp(p1, (c0,c2)) * bilerp(p2, (c0,c1))
    out[n, 16:32] = linelerp(line0, x[:, 3])

The SWDGE descriptor generation on the Pool engine (~2.8 ns/descriptor,
serialized) is the bottleneck, so only grid3d and plane1 are fetched via
dma_gather (2 descriptors/point instead of 4).  plane0 and plane2 are
evaluated on the otherwise-idle PE: points are sorted into (gy, gx-pair)
windows; within a window plane0 touches <= 4x18 positions and plane2
<= 4x6, so a per-window stationary [96, 32] (p0 patch ++ p2 patch ->
16+16 output channels) times a host-built sparse-weight moving tile
computes both bilerps for 32-point blocks.  The PSUM result (column
layout) returns to the row layout via an HWDGE dma transpose, and the
final product is a pair of cheap row-layout DVE multiplies.

  - Host: compute cell indices + per-corner lerp weight products, route
    points to the 8 cores by grid z-slab, sort by window, pad windows to
    32-point blocks, shuffle blocks (spreads gather addresses; avoids the
    HBM locality cliff of fully sorted gathers), and build fp16 dup-block
    tables whose rows hold a full interpolation neighborhood:
      grid:   (16ch, 8 corners) fp16 = 256B per row, 8*64*64 rows per slab
      plane1: (16ch, 4 corners) fp16 = 128B + 128B pad, 128*128 rows
  - Device (per chunk of 128*S points): fp16 corner weights + wrapped
    int16 indices, 2x2 SWDGE dma_gather, per-block PE matmuls for p0/p2,
    fp16 DVE weighted combine, store (128, S, 16) fp16 (host upcasts).
  - Line lerp on PE: stationary (64, 16) line table, moving host-built
    (64, CHUNK) sparse weight columns, PSUM -> fp16 via the scalar engine.
"""

import math
import numpy as np

import concourse.bacc as bacc
import concourse.bass as bass
import concourse.tile as tile
from concourse import mybir
from concourse import bass_utils

# ---------------- problem constants (hardcoded) ----------------
N = 1_000_000
C = 16
D = H = W = 128        # grid3d spatial dims
HP = WP = 256          # plane dims
LL = 64                # line length
NCORES = 8

S = 32                 # point-groups per partition per chunk
CHUNK = 128 * S        # points per chunk
NW = 12                # weight columns per point (8 grid + 4 plane1)
BLK = 32               # points per p0/p2 matmul block
NBLK = CHUNK // BLK    # 128 blocks per chunk
KP0 = 72               # plane0 stationary rows: 4 py * 18 pz
KP2 = 24               # plane2 stationary rows: 4 py * 6 px
KST = KP0 + KP2        # 96
NWIN = 64 * 32         # (gy, gx-pair) windows per core

F32 = mybir.dt.float32
F16 = mybir.dt.float16
I16 = mybir.dt.int16


# ---------------- walrus / tile workarounds ----------------
_PATCHED = False


def _apply_patches():
    """This container's walrus rejects >1 sync-wait command on the Tile tail
    drain; split the waits into explicit wait_ge instructions."""
    global _PATCHED
    if _PATCHED:
        return
    _PATCHED = True
    import concourse.tile as tile_mod
    from concourse.tile import ScopedClock

    def _drain_and_barrier_split(self, tick_clock, wait_clock):
        drain_inst = self.nc.sync.drain()
        wait_clock.add_sem_waits(
            drain_inst.ins, ScopedClock({None: tick_clock.global_clock})
        )
        si = drain_inst.ins.sync_info
        if si is not None and len(si.on_wait) > 1:
            assert self.sems is not None
            by_name = {h.name: h for h in self.sems.allocated().values()}
            keep, spill = [], []
            for w in si.on_wait:
                h = by_name.get(w.ant_name)
                if h is None or len(keep) < 1:
                    keep.append(w)
                else:
                    spill.append((h, w.wait_value))
            si.on_wait = keep
            for h, v in spill:
                self.nc.sync.wait_ge(h, v)
        self.nc.all_engine_barrier()
        assert self.sems is not None
        popped = self.nc._tile_sem_poison_stack.pop()
        assert popped is self._sem_poison
        self.nc.clear_and_free_semaphores(list(self.sems.allocated().values()))
        self.nc.all_engine_barrier()

    tile_mod.TileContext._drain_and_barrier = _drain_and_barrier_split


# ---------------- device program ----------------

def build_program(nchunks: int):
    """Build + compile the SPMD bass program for `nchunks` chunks per core."""
    _apply_patches()
    nc = bacc.Bacc(
        "TRN2",
        num_devices=1,
        debug=False,
        target_bir_lowering=False,
        num_swdge_queues=4,
    )
    FS = S * 8   # idx cols per partition

    wts_d = nc.dram_tensor("wts", (nchunks, 128, S * NW), F16, kind="ExternalInput").ap()
    ig_d = nc.dram_tensor("idxg", (nchunks, 128, FS), I16, kind="ExternalInput").ap()
    ip1_d = nc.dram_tensor("idxp1", (nchunks, 128, FS), I16, kind="ExternalInput").ap()
    al_d = nc.dram_tensor("al", (nchunks, 64, CHUNK), F16, kind="ExternalInput").ap()
    w0_d = nc.dram_tensor("w0", (nchunks, KST, CHUNK), F16, kind="ExternalInput").ap()
    st_d = nc.dram_tensor("st", (nchunks, KST, NBLK * BLK), F16, kind="ExternalInput").ap()
    gtab = nc.dram_tensor("gtab", (8 * 64 * 64, 128), F16, kind="ExternalInput").ap()
    p1tab = nc.dram_tensor("p1tab", (128 * 128, 128), F16, kind="ExternalInput").ap()
    ltab = nc.dram_tensor("ltab", (LL, C), F16, kind="ExternalInput").ap()
    out_d = nc.dram_tensor("out", (nchunks, 128, S * 16), F16, kind="ExternalOutput").ap()
    oln_d = nc.dram_tensor("oln", (nchunks, 16, CHUNK), F16, kind="ExternalOutput").ap()

    mul = mybir.AluOpType.mult
    add = mybir.AluOpType.add

    with tile.TileContext(nc) as tc:
        with tc.tile_pool(name="pconst", bufs=1) as pconst, \
             tc.tile_pool(name="pin", bufs=3) as pin, \
             tc.tile_pool(name="pval", bufs=4) as pval, \
             tc.tile_pool(name="ptmp", bufs=2) as ptmp, \
             tc.tile_pool(name="pps", bufs=4, space="PSUM") as pps, \
             tc.tile_pool(name="pps2", bufs=4, space="PSUM") as pps2, \
             tc.tile_pool(name="pout", bufs=2) as pout:
            lsb = pconst.tile([LL, C], F16, tag="lsb")
            nc.sync.dma_start(out=lsb[:], in_=ltab)

            for k in range(nchunks):
                # ---- loads ----
                wts = pin.tile([128, S, NW], F16, tag="wts")
                nc.sync.dma_start(out=wts[:], in_=wts_d[k].rearrange("p (s q) -> p s q", q=NW))
                ig = pin.tile([128, FS], I16, tag="ig")
                nc.sync.dma_start(out=ig[:], in_=ig_d[k])
                ip1 = pin.tile([128, FS], I16, tag="ip1")
                nc.sync.dma_start(out=ip1[:], in_=ip1_d[k])
                al = pin.tile([64, CHUNK], F16, tag="al")
                nc.sync.dma_start(out=al[:], in_=al_d[k])
                w0t = pin.tile([KST, CHUNK], F16, tag="w0t")
                nc.sync.dma_start(out=w0t[:], in_=w0_d[k])
                stt = pin.tile([KST, NBLK * BLK], F16, tag="stt")
                nc.sync.dma_start(out=stt[:], in_=st_d[k])

                # ---- gathers (grid + plane1; one table per SWDGE queue,
                # each split into two half-chunk gathers issued round-robin
                # so the queues stay fed across chunk boundaries) ----
                vg = pval.tile([128, S, 128], F16, tag="vg")
                vp1 = pval.tile([128, S, 128], F16, tag="vp1")
                tabs = ((vg, gtab, ig), (vp1, p1tab, ip1))
                HS, HC, HF = S // 2, CHUNK // 2, FS // 2
                for h in range(2):
                    for v, tab, ip in tabs:
                        nc.gpsimd.dma_gather(
                            v[:, HS * h:HS * (h + 1), :], tab,
                            ip[:, HF * h:HF * (h + 1)], HC, HC, 128,
                            queue_num=0, single_packet=False)

                # ---- line lerp on PE: (64,16)^T @ (64,512) x 8 ----
                oln = pout.tile([16, CHUNK], F16, tag="oln")
                for j in range(CHUNK // 512):
                    ps = pps.tile([16, 512], F32, tag="ps")
                    nc.tensor.matmul(ps[:], lhsT=lsb[:], rhs=al[:, 512 * j:512 * (j + 1)])
                    nc.scalar.copy(out=oln[:, 512 * j:512 * (j + 1)], in_=ps[:])
                nc.scalar.dma_start(out=oln_d[k], in_=oln[:])

                # ---- plane0 + plane2 on PE: per-block [96,32] stationary ----
                p02sb = ptmp.tile([32, CHUNK], F16, tag="p02sb")
                for g in range(CHUNK // 512):
                    ps2 = pps2.tile([32, 512], F32, tag="ps2")
                    for b in range(512 // BLK):
                        blk = g * (512 // BLK) + b
                        nc.tensor.matmul(
                            ps2[:, BLK * b:BLK * (b + 1)],
                            lhsT=stt[:, BLK * blk:BLK * (blk + 1)],
                            rhs=w0t[:, BLK * blk:BLK * (blk + 1)],
                            start=True, stop=True)
                    nc.scalar.copy(out=p02sb[:, 512 * g:512 * (g + 1)], in_=ps2[:])
                p02row = ptmp.tile([128, S, 32], F16, tag="p02row")
                nc.scalar.dma_start_transpose(out=p02row[:], in_=p02sb[:])

                out_t = pout.tile([128, S, 16], F16, tag="out")
                out16 = out_t[:, :, 0:16]

                # ---- combine: grid (row = 16ch x 8 corners, fp16 2x mode) ----
                vg4 = vg[:].rearrange("p s (c k) -> p s c k", k=8)
                nc.vector.tensor_tensor(
                    out=vg4, in0=vg4,
                    in1=wts[:, :, 0:8].unsqueeze(2).broadcast_to([128, S, 16, 8]),
                    op=mul,
                )
                nc.vector.tensor_tensor(
                    out=vg4[:, :, :, 0:4], in0=vg4[:, :, :, 0:4], in1=vg4[:, :, :, 4:8], op=add)
                nc.vector.tensor_tensor(
                    out=vg4[:, :, :, 0:2], in0=vg4[:, :, :, 0:2], in1=vg4[:, :, :, 2:4], op=add)
                nc.vector.tensor_tensor(
                    out=out16, in0=vg4[:, :, :, 0], in1=vg4[:, :, :, 1], op=add)

                # ---- combine: plane1 (row = 16ch x 4 corners + pad) ----
                v4 = vp1[:, :, 0:64].rearrange("p s (c k) -> p s c k", k=4)
                nc.vector.tensor_tensor(
                    out=v4, in0=v4,
                    in1=wts[:, :, 8:12].unsqueeze(2).broadcast_to([128, S, 16, 4]),
                    op=mul,
                )
                nc.vector.tensor_tensor(
                    out=v4[:, :, :, 0:2], in0=v4[:, :, :, 0:2], in1=v4[:, :, :, 2:4],
                    op=add)
                tsum = ptmp.tile([128, S, 16], F16, tag="ts1")
                nc.vector.tensor_tensor(
                    out=tsum[:], in0=v4[:, :, :, 0], in1=v4[:, :, :, 1], op=add)
                nc.vector.tensor_tensor(
                    out=out16, in0=out16, in1=tsum[:], op=mul)

                # ---- multiply in plane0 / plane2 (row layout) ----
                nc.vector.tensor_tensor(
                    out=out16, in0=out16, in1=p02row[:, :, 0:16], op=mul)
                nc.vector.tensor_tensor(
                    out=out16, in0=out16, in1=p02row[:, :, 16:32], op=mul)

                # ---- store ----
                nc.scalar.dma_start(out=out_d[k], in_=out_t[:].rearrange("p s q -> p (s q)"))

    # Spread gathers across the 4 SWDGE queues.  Tile assigned each Pool-DMA
    # a DMASW{lane} sem in scheduled order; a sem must always be fed by the
    # same queue, so derive queue_num = lane % 4.
    for bb in nc.m.functions[0].blocks:
        for inst in bb.instructions:
            if isinstance(inst, mybir.InstDMAGatherAnt):
                si = inst.sync_info
                for u in (si.on_update if si else []):
                    if u.ant_name.startswith("DMASW"):
                        lane = int(u.ant_name[5:].split("_")[0])
                        inst.queue_num = lane % 4
                        break
    nc.compile()
    return nc


_PROGRAM_CACHE = {}


def _get_program(nchunks: int):
    if nchunks not in _PROGRAM_CACHE:
        _PROGRAM_CACHE[nchunks] = build_program(nchunks)
    return _PROGRAM_CACHE[nchunks]


# ---------------- host-side preparation ----------------

def _split_idx_host(p, lo, hi):
    """Clamped floor + weight, matching the reference within [lo, hi+1]."""
    i0 = np.clip(np.floor(p), lo, hi).astype(np.int32)
    w = (p - i0.astype(np.float32)).astype(np.float32)
    return i0, w


def _build_tables(grid3d, plane1):
    gT = np.ascontiguousarray(grid3d.transpose(1, 2, 3, 0)).astype(np.float16)  # (D,H,W,C)
    # per-core z-slab dup-block tables: core c owns z-origins 63+8c .. 63+8c+7
    # row layout: (16 ch, 8 corners) with corner = 4dz+2dy+dx
    gtabs = []
    for c in range(NCORES):
        z0 = 63 + 8 * c
        blk = np.empty((8, 64, 64, C, 2, 2, 2), np.float16)
        for dz in range(2):
            for dy in range(2):
                for dx in range(2):
                    blk[:, :, :, :, dz, dy, dx] = gT[
                        z0 + dz:z0 + dz + 8, 63 + dy:127 + dy, 63 + dx:127 + dx, :]
        gtabs.append(blk.reshape(8 * 64 * 64, 128))

    # plane1 rows: elems 0:64 = (16 ch, 4 corners) with corner = 2dy+dx
    pT = np.ascontiguousarray(plane1.transpose(1, 2, 0)).astype(np.float16)  # (H,W,C)
    blk = np.zeros((128, 128, 128), np.float16)
    core = blk[:, :, 0:64].reshape(128, 128, C, 2, 2)
    for dy in range(2):
        for dx in range(2):
            core[:, :, :, dy, dx] = pT[127 + dy:255 + dy, 127 + dx:255 + dx, :]
    p1t = blk.reshape(128 * 128, 128)
    return gtabs, p1t


def _wrap_idx(idx_slot, nchunks):
    """(cap,) int in slot order -> (nchunks, 128, 8S) int16 wrapped layout."""
    a = idx_slot.astype(np.int16).reshape(nchunks, S, 8, 16)
    a = a.transpose(0, 3, 1, 2).reshape(nchunks, 16, 8 * S)
    return np.ascontiguousarray(np.tile(a, (1, 8, 1)))


def _corner_weights(wgh, w1y, w1x):
    """(npts, NW) f16: grid 8 corner products + plane1 4 corner products."""
    npts = wgh.shape[0]
    wts = np.zeros((npts, NW), np.float16)
    one = np.float32(1.0)
    wx, wy, wz = wgh[:, 0], wgh[:, 1], wgh[:, 2]
    zs = (one - wz, wz)
    ys = (one - wy, wy)
    xs = (one - wx, wx)
    for dz in range(2):
        for dy in range(2):
            zy = zs[dz] * ys[dy]
            for dx in range(2):
                wts[:, 4 * dz + 2 * dy + dx] = zy * xs[dx]
    # plane1 ((c0,c2) -> W from x, H from z): corner = 2*dH + dW
    cys = (one - w1y, w1y)
    cxs = (one - w1x, w1x)
    for dy in range(2):
        for dx in range(2):
            wts[:, 8 + 2 * dy + dx] = cys[dy] * cxs[dx]
    return wts


def kernel(x, grid3d, plane0, plane1, plane2, line0):
    x = np.asarray(x, np.float32)
    grid3d = np.asarray(grid3d, np.float32)
    plane0 = np.asarray(plane0, np.float32)
    plane1 = np.asarray(plane1, np.float32)
    plane2 = np.asarray(plane2, np.float32)
    line0 = np.asarray(line0, np.float32)

    npts_total = x.shape[0]
    half = np.float32(0.5)
    one = np.float32(1.0)

    # coordinates in the reference's f32 arithmetic order
    pg = ((x[:, 0:3] + one) * half) * np.float32(D - 1)   # grid:  coords 0,1,2
    pp = ((x[:, 0:3] + one) * half) * np.float32(HP - 1)  # plane coords
    pl = x[:, 3] * np.float32(LL - 1)

    i0g, wgh = _split_idx_host(pg, 63, 126)
    i0p, wph = _split_idx_host(pp, 127, 254)
    i0l, wlh = _split_idx_host(pl, 0, 62)

    # z-slab routing (grid z = coord 2)
    slab = (i0g[:, 2] - 63) >> 3

    # window key within core: (gy_local, gx_pair)
    gyl = i0g[:, 1] - 63          # 0..63
    gxp = (i0g[:, 0] - 63) >> 1   # 0..31
    wkey = gyl * 32 + gxp         # 0..2047

    # per-point table indices (slab-local grid)
    idx_g = ((i0g[:, 2] - 63 - 8 * slab) * 64 + (i0g[:, 1] - 63)) * 64 + (i0g[:, 0] - 63)
    idx_p1 = (i0p[:, 2] - 127) * 128 + (i0p[:, 0] - 127)

    # plane1 weights: H from z (coord 2), W from x (coord 0)
    wts = _corner_weights(wgh, wph[:, 2], wph[:, 0])
    wl16 = wlh.astype(np.float16)
    wl16c = (np.float32(1.0) - wlh).astype(np.float16)

    gtabs, p1t = _build_tables(grid3d, plane1)
    ltab = np.ascontiguousarray(line0.T).astype(np.float16)  # (L, C)

    # pad planes for stationary-patch builds (indices clipped below anyway)
    p0pad = plane0.astype(np.float16)
    p2pad = plane2.astype(np.float16)

    rng = np.random.default_rng(1234)

    # ---- per-core window packing (two passes: sizes, then streams) ----
    core_data = []
    max_padded = 0
    for c in range(NCORES):
        selc = np.nonzero(slab == c)[0]
        wk = wkey[selc]
        order = np.argsort(wk, kind="stable")
        selc = selc[order]
        wk = wk[order]
        counts = np.bincount(wk, minlength=NWIN)
        padded = ((counts + BLK - 1) // BLK) * BLK
        offs = np.zeros(NWIN + 1, np.int64)
        offs[1:] = np.cumsum(padded)
        total = int(offs[-1])
        max_padded = max(max_padded, total)
        core_data.append((selc, wk, counts, padded, offs))

    nchunks = max(1, math.ceil(max_padded / CHUNK))
    cap = nchunks * CHUNK
    nblk_cap = cap // BLK

    in_maps = []
    sel_blocked_all = []
    for c in range(NCORES):
        selc, wk, counts, padded, offs = core_data[c]
        total = int(offs[-1])

        # window-first-point fill, then scatter real points to their slots
        first_of_win = np.zeros(NWIN, np.int64)
        nz = counts > 0
        # first occurrence index in selc for each nonempty window
        starts = np.zeros(NWIN + 1, np.int64)
        starts[1:] = np.cumsum(counts)
        first_of_win[nz] = selc[starts[:-1][nz]]

        sel_blocked = np.repeat(first_of_win, padded)
        win_of_slot = np.repeat(np.arange(NWIN), padded)
        # rank of each sorted point within its window
        rank = np.arange(selc.shape[0], dtype=np.int64) - np.repeat(starts[:-1], counts)
        slot = offs[:-1][wk] + rank
        sel_blocked[slot] = selc
        # tail fill: replicate the first block
        if cap > total:
            pad_n = cap - total
            if total >= BLK:
                sel_blocked = np.concatenate(
                    [sel_blocked, np.tile(sel_blocked[:BLK], pad_n // BLK)])
                win_of_slot = np.concatenate(
                    [win_of_slot, np.tile(win_of_slot[:BLK], pad_n // BLK)])
            else:
                sel_blocked = np.concatenate(
                    [sel_blocked, np.repeat(sel_blocked[:1] if total else [0], pad_n)])
                win_of_slot = np.concatenate(
                    [win_of_slot, np.repeat(win_of_slot[:1] if total else [0], pad_n)])

        # shuffle blocks to spread gather addresses
        perm = rng.permutation(nblk_cap)
        sel_blocked = sel_blocked.reshape(nblk_cap, BLK)[perm].reshape(cap)
        win_of_slot = win_of_slot.reshape(nblk_cap, BLK)[perm].reshape(cap)
        win_of_block = win_of_slot.reshape(nblk_cap, BLK)[:, 0]

        # ---- window bases (empirical minima; exact, no ulp hazards) ----
        pz_pt = i0p[selc, 2]
        py_pt = i0p[selc, 1]
        px_pt = i0p[selc, 0]
        big = np.full(NWIN, 32767, np.int32)
        pzb = big.copy(); pyb = big.copy(); pxb = big.copy()
        np.minimum.at(pzb, wk, pz_pt)
        np.minimum.at(pyb, wk, py_pt)
        np.minimum.at(pxb, wk, px_pt)
        # verify spans fit the stationary patches
        mx = np.zeros(NWIN, np.int32)
        np.maximum.at(mx, wk, pz_pt)
        assert ((mx - pzb)[nz] <= 16).all(), "pz span too large"
        mx[:] = 0
        np.maximum.at(mx, wk, py_pt)
        assert ((mx - pyb)[nz] <= 2).all(), "py span too large"
        mx[:] = 0
        np.maximum.at(mx, wk, px_pt)
        assert ((mx - pxb)[nz] <= 4).all(), "px span too large"
        pzb[~nz] = 127; pyb[~nz] = 127; pxb[~nz] = 127

        # ---- per-window stationaries [KST, 32] ----
        stat_win = np.zeros((NWIN, KST, 32), np.float16)
        for dz in range(18):
            zi = np.minimum(pzb + dz, 255)
            for dy in range(4):
                yi = np.minimum(pyb + dy, 255)
                stat_win[:, dz * 4 + dy, 0:16] = p0pad[:, zi, yi].T
        for dy in range(4):
            yi = np.minimum(pyb + dy, 255)
            for dx in range(6):
                xi = np.minimum(pxb + dx, 255)
                stat_win[:, KP0 + dy * 6 + dx, 16:32] = p2pad[:, yi, xi].T
        stat_blocks = stat_win[win_of_block]                     # (nblk_cap, KST, 32)
        st = stat_blocks.reshape(nchunks, NBLK, KST, BLK)
        st = np.ascontiguousarray(st.transpose(0, 2, 1, 3).reshape(nchunks, KST, NBLK * BLK))

        # ---- sparse weight columns W0 [cap, KST] ----
        pidx = sel_blocked
        wlot = win_of_slot
        z0o = (i0p[pidx, 2] - pzb[wlot]).astype(np.int64)
        y0o = (i0p[pidx, 1] - pyb[wlot]).astype(np.int64)
        x0o = (i0p[pidx, 0] - pxb[wlot]).astype(np.int64)
        wz1 = wph[pidx, 2]; wy1 = wph[pidx, 1]; wx1 = wph[pidx, 0]
        one = np.float32(1.0)
        W0 = np.zeros((cap, KST), np.float16)
        ar = np.arange(cap)
        for dz in range(2):
            vz = (one - wz1) if dz == 0 else wz1
            for dy in range(2):
                vy = (one - wy1) if dy == 0 else wy1
                W0[ar, (z0o + dz) * 4 + (y0o + dy)] = vz * vy
        for dy in range(2):
            vy = (one - wy1) if dy == 0 else wy1
            for dx in range(2):
                vx = (one - wx1) if dx == 0 else wx1
                W0[ar, KP0 + (y0o + dy) * 6 + (x0o + dx)] = vy * vx
        w0 = np.ascontiguousarray(
            W0.reshape(nchunks, CHUNK, KST).transpose(0, 2, 1))

        # ---- gather weights / indices (slot i == blocked position i:
        # both the gather and the dma transpose map position i to
        # (partition i%128, group i//128)) ----
        wtsc = wts[pidx].reshape(nchunks, S, 128, NW).transpose(0, 2, 1, 3)
        wtsc = np.ascontiguousarray(wtsc.reshape(nchunks, 128, S * NW))
        idxg_slot = idx_g[pidx]
        idxp1_slot = idx_p1[pidx]

        # ---- line lerp columns (blocked-sorted order) ----
        alc = np.zeros((cap, LL), np.float16)
        i0s = i0l[pidx]
        alc[ar, i0s] = wl16c[pidx]
        alc[ar, i0s + 1] = wl16[pidx]
        alc = np.ascontiguousarray(alc.reshape(nchunks, CHUNK, LL).transpose(0, 2, 1))

        in_maps.append({
            "wts": wtsc,
            "idxg": _wrap_idx(idxg_slot, nchunks),
            "idxp1": _wrap_idx(idxp1_slot, nchunks),
            "al": alc,
            "w0": w0,
            "st": st,
            "gtab": gtabs[c],
            "p1tab": p1t,
            "ltab": ltab,
        })
        sel_blocked_all.append(sel_blocked)

    nc = _get_program(nchunks)
    res = bass_utils.run_bass_kernel_spmd(nc, in_maps, core_ids=list(range(NCORES)))
    kernel.last_results = res

    out = np.empty((npts_total, 32), np.float32)
    for c in range(NCORES):
        sel_blocked = sel_blocked_all[c]
        # out tile (k, p, s, 16): slot (p, s) holds blocked position s*128+p
        o = res.results[c]["out"].astype(np.float32).reshape(nchunks, 128, S, 16)
        o = o.transpose(0, 2, 1, 3).reshape(cap, 16)
        out[sel_blocked, 0:16] = o
        ol = res.results[c]["oln"].astype(np.float32)  # (nchunks, 16, CHUNK)
        ol = ol.transpose(0, 2, 1).reshape(cap, 16)
        out[sel_blocked, 16:32] = ol
    return out


# revision 10
# speedup vs baseline: 1.5876x; 1.4615x over previous
"""Trainium2 Bass kernel for nn_DecompGrid (factorized-grid embedding lookup).

Computation (per point, C=16 channels):
    out[n, 0:16]  = trilerp(grid3d, xyz) * bilerp(p0, (c1,c2)) * bilerp(p1, (c0,c2)) * bilerp(p2, (c0,c1))
    out[n, 16:32] = linelerp(line0, x[:, 3])

Strategy:
  - Host: compute cell indices + per-corner lerp weight PRODUCTS (cheap
    vectorized numpy), route points to the 8 cores by grid z-slab so the
    per-core grid table fits the dma_gather int16 index limit (<= 32768 rows),
    and build fp16 "dup-block" tables whose rows hold a full interpolation
    neighborhood in channel-major order (unit-stride inner dims for the DVE
    2x fp16 perf mode):
      grid:  (16ch, 8 corners) fp16 = 256B per row, 8*64*64 rows per core slab
      plane: (16ch, 4 corners) fp16 = 128B + 128B pad,  128*128 rows
  - Device (per chunk of 128*S points): load fp16 corner weights + wrapped
    int16 indices, 4x SWDGE dma_gather (one row per point per table, one table
    per SWDGE queue so the drains balance), fp16 DVE weighted combine, store
    (128, S, 16) fp16 spatial (host upcasts).
  - The line lerp runs on the otherwise-idle PE: stationary = (64, 16) line
    table, moving = host-built (64, CHUNK) sparse weight columns (1-w at i0,
    w at i0+1), PSUM -> fp16 via the scalar engine, stored as (16, CHUNK).

The hot loop is bound by the gather descriptor drain (~4 rows/point).
"""

import math
import numpy as np

import concourse.bacc as bacc
import concourse.bass as bass
import concourse.tile as tile
from concourse import mybir
from concourse import bass_utils

# ---------------- problem constants (hardcoded) ----------------
N = 1_000_000
C = 16
D = H = W = 128        # grid3d spatial dims
HP = WP = 256          # plane dims
LL = 64                # line length
NCORES = 8

S = 32                 # point-groups per partition per chunk
CHUNK = 128 * S        # points per chunk
NW = 24                # weight columns per point (8 grid + 12 plane + pad)

F32 = mybir.dt.float32
F16 = mybir.dt.float16
I16 = mybir.dt.int16


# ---------------- walrus / tile workarounds ----------------
_PATCHED = False


def _apply_patches():
    """This container's walrus rejects >1 sync-wait command on the Tile tail
    drain; split the waits into explicit wait_ge instructions."""
    global _PATCHED
    if _PATCHED:
        return
    _PATCHED = True
    import concourse.tile as tile_mod
    from concourse.tile import ScopedClock

    def _drain_and_barrier_split(self, tick_clock, wait_clock):
        drain_inst = self.nc.sync.drain()
        wait_clock.add_sem_waits(
            drain_inst.ins, ScopedClock({None: tick_clock.global_clock})
        )
        si = drain_inst.ins.sync_info
        if si is not None and len(si.on_wait) > 1:
            assert self.sems is not None
            by_name = {h.name: h for h in self.sems.allocated().values()}
            keep, spill = [], []
            for w in si.on_wait:
                h = by_name.get(w.ant_name)
                if h is None or len(keep) < 1:
                    keep.append(w)
                else:
                    spill.append((h, w.wait_value))
            si.on_wait = keep
            for h, v in spill:
                self.nc.sync.wait_ge(h, v)
        self.nc.all_engine_barrier()
        assert self.sems is not None
        popped = self.nc._tile_sem_poison_stack.pop()
        assert popped is self._sem_poison
        self.nc.clear_and_free_semaphores(list(self.sems.allocated().values()))
        self.nc.all_engine_barrier()

    tile_mod.TileContext._drain_and_barrier = _drain_and_barrier_split


# ---------------- device program ----------------

def build_program(nchunks: int, single_packet: bool = False):
    """Build + compile the SPMD bass program for `nchunks` chunks per core."""
    _apply_patches()
    nc = bacc.Bacc(
        "TRN2",
        num_devices=1,
        debug=False,
        target_bir_lowering=False,
        num_swdge_queues=4,
    )
    FS = S * 8   # idx cols per partition

    wts_d = nc.dram_tensor("wts", (nchunks, 128, S * NW), F16, kind="ExternalInput").ap()
    ig_d = nc.dram_tensor("idxg", (nchunks, 128, FS), I16, kind="ExternalInput").ap()
    ip0_d = nc.dram_tensor("idxp0", (nchunks, 128, FS), I16, kind="ExternalInput").ap()
    ip1_d = nc.dram_tensor("idxp1", (nchunks, 128, FS), I16, kind="ExternalInput").ap()
    ip2_d = nc.dram_tensor("idxp2", (nchunks, 128, FS), I16, kind="ExternalInput").ap()
    al_d = nc.dram_tensor("al", (nchunks, 64, CHUNK), F16, kind="ExternalInput").ap()
    gtab = nc.dram_tensor("gtab", (8 * 64 * 64, 128), F16, kind="ExternalInput").ap()
    p0tab = nc.dram_tensor("p0tab", (128 * 128, 128), F16, kind="ExternalInput").ap()
    p1tab = nc.dram_tensor("p1tab", (128 * 128, 128), F16, kind="ExternalInput").ap()
    p2tab = nc.dram_tensor("p2tab", (128 * 128, 128), F16, kind="ExternalInput").ap()
    ltab = nc.dram_tensor("ltab", (LL, C), F16, kind="ExternalInput").ap()
    out_d = nc.dram_tensor("out", (nchunks, 128, S * 16), F16, kind="ExternalOutput").ap()
    oln_d = nc.dram_tensor("oln", (nchunks, 16, CHUNK), F16, kind="ExternalOutput").ap()

    mul = mybir.AluOpType.mult
    add = mybir.AluOpType.add

    with tile.TileContext(nc) as tc:
        with tc.tile_pool(name="pconst", bufs=1) as pconst, \
             tc.tile_pool(name="pin", bufs=3) as pin, \
             tc.tile_pool(name="pval", bufs=4) as pval, \
             tc.tile_pool(name="ptmp", bufs=2) as ptmp, \
             tc.tile_pool(name="pps", bufs=4, space="PSUM") as pps, \
             tc.tile_pool(name="pout", bufs=2) as pout:
            lsb = pconst.tile([LL, C], F16, tag="lsb")
            nc.sync.dma_start(out=lsb[:], in_=ltab)

            for k in range(nchunks):
                # ---- loads ----
                wts = pin.tile([128, S, NW], F16, tag="wts")
                nc.sync.dma_start(out=wts[:], in_=wts_d[k].rearrange("p (s q) -> p s q", q=NW))
                ig = pin.tile([128, FS], I16, tag="ig")
                nc.sync.dma_start(out=ig[:], in_=ig_d[k])
                ip0 = pin.tile([128, FS], I16, tag="ip0")
                nc.sync.dma_start(out=ip0[:], in_=ip0_d[k])
                ip1 = pin.tile([128, FS], I16, tag="ip1")
                nc.sync.dma_start(out=ip1[:], in_=ip1_d[k])
                ip2 = pin.tile([128, FS], I16, tag="ip2")
                nc.sync.dma_start(out=ip2[:], in_=ip2_d[k])
                al = pin.tile([64, CHUNK], F16, tag="al")
                nc.sync.dma_start(out=al[:], in_=al_d[k])

                # ---- gathers (one table per SWDGE queue; each split into two
                # half-chunk gathers issued round-robin across tables, so a
                # context-wait stall on the GpSimd engine covers only half a
                # drain and the queues stay fed across chunk boundaries) ----
                vg = pval.tile([128, S, 128], F16, tag="vg")
                vps = [pval.tile([128, S, 128], F16, tag=f"vp{t}", name=f"vp{t}")
                       for t in range(3)]
                tabs = ((vg, gtab, ig), (vps[0], p0tab, ip0),
                        (vps[1], p1tab, ip1), (vps[2], p2tab, ip2))
                for v, tab, ip in tabs:
                    nc.gpsimd.dma_gather(
                        v[:], tab, ip[:], CHUNK, CHUNK, 128,
                        queue_num=0, single_packet=single_packet)

                # ---- line lerp on PE: (64,16)^T @ (64,512) x 8 ----
                oln = pout.tile([16, CHUNK], F16, tag="oln")
                for j in range(CHUNK // 512):
                    ps = pps.tile([16, 512], F32, tag="ps")
                    nc.tensor.matmul(ps[:], lhsT=lsb[:], rhs=al[:, 512 * j:512 * (j + 1)])
                    nc.scalar.copy(out=oln[:, 512 * j:512 * (j + 1)], in_=ps[:])
                nc.sync.dma_start(out=oln_d[k], in_=oln[:])

                out_t = pout.tile([128, S, 16], F16, tag="out")
                out16 = out_t[:, :, 0:16]

                # ---- combine: grid (row = 16ch x 8 corners, fp16 2x mode) ----
                vg4 = vg[:].rearrange("p s (c k) -> p s c k", k=8)
                nc.vector.tensor_tensor(
                    out=vg4, in0=vg4,
                    in1=wts[:, :, 0:8].unsqueeze(2).broadcast_to([128, S, 16, 8]),
                    op=mul,
                )
                nc.vector.tensor_tensor(
                    out=vg4[:, :, :, 0:4], in0=vg4[:, :, :, 0:4], in1=vg4[:, :, :, 4:8], op=add)
                nc.vector.tensor_tensor(
                    out=vg4[:, :, :, 0:2], in0=vg4[:, :, :, 0:2], in1=vg4[:, :, :, 2:4], op=add)
                nc.vector.tensor_tensor(
                    out=out16, in0=vg4[:, :, :, 0], in1=vg4[:, :, :, 1], op=add)

                # ---- combine: planes (row = 16ch x 4 corners + pad) ----
                for t, v in enumerate(vps):
                    v4 = v[:, :, 0:64].rearrange("p s (c k) -> p s c k", k=4)
                    nc.vector.tensor_tensor(
                        out=v4, in0=v4,
                        in1=wts[:, :, 8 + 4 * t: 12 + 4 * t].unsqueeze(2)
                               .broadcast_to([128, S, 16, 4]),
                        op=mul,
                    )
                    nc.vector.tensor_tensor(
                        out=v4[:, :, :, 0:2], in0=v4[:, :, :, 0:2], in1=v4[:, :, :, 2:4],
                        op=add)
                    tsum = ptmp.tile([128, S, 16], F16, tag=f"ts{t}")
                    nc.vector.tensor_tensor(
                        out=tsum[:], in0=v4[:, :, :, 0], in1=v4[:, :, :, 1], op=add)
                    nc.vector.tensor_tensor(
                        out=out16, in0=out16, in1=tsum[:], op=mul)

                # ---- store ----
                nc.sync.dma_start(out=out_d[k], in_=out_t[:].rearrange("p s q -> p (s q)"))

    # Spread gathers across the 4 SWDGE queues (4 Q7 core pairs generate
    # descriptors in parallel). Tile assigned each Pool-DMA a DMASW{lane} sem
    # in scheduled order; a sem must always be fed by the same queue, so
    # derive queue_num = lane % 4.
    for bb in nc.m.functions[0].blocks:
        for inst in bb.instructions:
            if isinstance(inst, mybir.InstDMAGatherAnt):
                si = inst.sync_info
                for u in (si.on_update if si else []):
                    if u.ant_name.startswith("DMASW"):
                        lane = int(u.ant_name[5:].split("_")[0])
                        inst.queue_num = lane % 4
                        break
    nc.compile()
    return nc


_PROGRAM_CACHE = {}


def _get_program(nchunks: int):
    if nchunks not in _PROGRAM_CACHE:
        _PROGRAM_CACHE[nchunks] = build_program(nchunks)
    return _PROGRAM_CACHE[nchunks]


# ---------------- host-side preparation ----------------

def _split_idx_host(p, lo, hi):
    """Clamped floor + weight, matching the reference within [lo, hi+1]."""
    i0 = np.clip(np.floor(p), lo, hi).astype(np.int32)
    w = (p - i0.astype(np.float32)).astype(np.float32)
    return i0, w


def _build_tables(grid3d, plane0, plane1, plane2, line0):
    gT = np.ascontiguousarray(grid3d.transpose(1, 2, 3, 0)).astype(np.float16)  # (D,H,W,C)
    # per-core z-slab dup-block tables: core c owns z-origins 63+8c .. 63+8c+7
    # row layout: (16 ch, 8 corners) with corner = 4dz+2dy+dx
    gtabs = []
    for c in range(NCORES):
        z0 = 63 + 8 * c
        blk = np.empty((8, 64, 64, C, 2, 2, 2), np.float16)
        for dz in range(2):
            for dy in range(2):
                for dx in range(2):
                    blk[:, :, :, :, dz, dy, dx] = gT[
                        z0 + dz:z0 + dz + 8, 63 + dy:127 + dy, 63 + dx:127 + dx, :]
        gtabs.append(blk.reshape(8 * 64 * 64, 128))

    # plane rows: elems 0:64 = (16 ch, 4 corners) with corner = 2dy+dx; 64:128 pad
    ptabs = []
    for plane in (plane0, plane1, plane2):
        pT = np.ascontiguousarray(plane.transpose(1, 2, 0)).astype(np.float16)  # (H,W,C)
        blk = np.zeros((128, 128, 128), np.float16)
        core = blk[:, :, 0:64].reshape(128, 128, C, 2, 2)
        for dy in range(2):
            for dx in range(2):
                core[:, :, :, dy, dx] = pT[127 + dy:255 + dy, 127 + dx:255 + dx, :]
        ptabs.append(blk.reshape(128 * 128, 128))

    ltab = np.ascontiguousarray(line0.T).astype(np.float16)  # (L, C)
    return gtabs, ptabs, ltab


def _wrap_idx(idx_sorted, nchunks):
    """(cap,) int -> (nchunks, 128, 8S) int16 wrapped dma_gather layout."""
    a = idx_sorted.astype(np.int16).reshape(nchunks, S, 8, 16)
    a = a.transpose(0, 3, 1, 2).reshape(nchunks, 16, 8 * S)
    return np.ascontiguousarray(np.tile(a, (1, 8, 1)))


def _corner_weights(wgh, wph):
    """(npts, NW) f16: per-corner weight products in gathered-row order."""
    npts = wgh.shape[0]
    wts = np.zeros((npts, NW), np.float16)
    one = np.float32(1.0)
    wx, wy, wz = wgh[:, 0], wgh[:, 1], wgh[:, 2]
    zs = (one - wz, wz)
    ys = (one - wy, wy)
    xs = (one - wx, wx)
    for dz in range(2):
        for dy in range(2):
            zy = zs[dz] * ys[dy]
            for dx in range(2):
                wts[:, 4 * dz + 2 * dy + dx] = zy * xs[dx]
    w0, w1, w2 = wph[:, 0], wph[:, 1], wph[:, 2]
    for t, (cy, cx) in enumerate(((w2, w1), (w2, w0), (w1, w0))):
        cys = (one - cy, cy)
        cxs = (one - cx, cx)
        for dy in range(2):
            for dx in range(2):
                wts[:, 8 + 4 * t + 2 * dy + dx] = cys[dy] * cxs[dx]
    return wts


def kernel(x, grid3d, plane0, plane1, plane2, line0):
    x = np.asarray(x, np.float32)
    grid3d = np.asarray(grid3d, np.float32)
    plane0 = np.asarray(plane0, np.float32)
    plane1 = np.asarray(plane1, np.float32)
    plane2 = np.asarray(plane2, np.float32)
    line0 = np.asarray(line0, np.float32)

    npts_total = x.shape[0]
    half = np.float32(0.5)
    one = np.float32(1.0)

    # coordinates in the reference's f32 arithmetic order
    pg = ((x[:, 0:3] + one) * half) * np.float32(D - 1)   # grid:  coords 0,1,2
    pp = ((x[:, 0:3] + one) * half) * np.float32(HP - 1)  # plane coords
    pl = x[:, 3] * np.float32(LL - 1)

    i0g, wgh = _split_idx_host(pg, 63, 126)
    i0p, wph = _split_idx_host(pp, 127, 254)
    i0l, wlh = _split_idx_host(pl, 0, 62)

    # z-slab routing (grid z = coord 2). Points stay in arrival order within
    # a slab on purpose: cell-sorting them concentrates the 16 SDMA engines'
    # concurrent gather reads onto the same HBM bank region and serializes
    # the drain (measured 1.7x slower); random order spreads banks/channels.
    slab = (i0g[:, 2] - 63) >> 3
    order = np.argsort(slab, kind="stable")
    counts = np.bincount(slab, minlength=NCORES)
    cap_pts = int(counts.max())
    nchunks = max(1, math.ceil(cap_pts / CHUNK))
    cap = nchunks * CHUNK

    # per-point table indices (slab-local grid)
    idx_g = ((i0g[:, 2] - 63 - 8 * slab) * 64 + (i0g[:, 1] - 63)) * 64 + (i0g[:, 0] - 63)
    idx_p0 = (i0p[:, 2] - 127) * 128 + (i0p[:, 1] - 127)
    idx_p1 = (i0p[:, 2] - 127) * 128 + (i0p[:, 0] - 127)
    idx_p2 = (i0p[:, 1] - 127) * 128 + (i0p[:, 0] - 127)

    wts = _corner_weights(wgh, wph)
    wl16 = wlh.astype(np.float16)
    wl16c = (np.float32(1.0) - wlh).astype(np.float16)

    gtabs, ptabs, ltab = _build_tables(grid3d, plane0, plane1, plane2, line0)

    offs = np.zeros(NCORES + 1, np.int64)
    offs[1:] = np.cumsum(counts)

    in_maps = []
    for c in range(NCORES):
        sel = order[offs[c]:offs[c + 1]]
        npts = sel.shape[0]
        pad = cap - npts
        if pad:
            sel = np.concatenate([sel, np.repeat(sel[:1] if npts else [0], pad)])

        wtsc = wts[sel].reshape(nchunks, S, 128, NW).transpose(0, 2, 1, 3)
        wtsc = np.ascontiguousarray(wtsc.reshape(nchunks, 128, S * NW))
        alc = np.zeros((cap, LL), np.float16)
        ar = np.arange(cap)
        i0s = i0l[sel]
        alc[ar, i0s] = wl16c[sel]
        alc[ar, i0s + 1] = wl16[sel]
        alc = np.ascontiguousarray(alc.reshape(nchunks, CHUNK, LL).transpose(0, 2, 1))
        in_maps.append({
            "wts": wtsc,
            "idxg": _wrap_idx(idx_g[sel], nchunks),
            "idxp0": _wrap_idx(idx_p0[sel], nchunks),
            "idxp1": _wrap_idx(idx_p1[sel], nchunks),
            "idxp2": _wrap_idx(idx_p2[sel], nchunks),
            "al": alc,
            "gtab": gtabs[c],
            "p0tab": ptabs[0],
            "p1tab": ptabs[1],
            "p2tab": ptabs[2],
            "ltab": ltab,
        })

    nc = _get_program(nchunks)
    res = bass_utils.run_bass_kernel_spmd(nc, in_maps, core_ids=list(range(NCORES)))
    kernel.last_results = res

    out = np.empty((npts_total, 32), np.float32)
    for c in range(NCORES):
        sel_c = order[offs[c]:offs[c + 1]]
        npts = int(counts[c])
        o = res.results[c]["out"].astype(np.float32).reshape(nchunks, 128, S, 16)
        o = o.transpose(0, 2, 1, 3).reshape(cap, 16)
        out[sel_c, 0:16] = o[:npts]
        ol = res.results[c]["oln"].astype(np.float32)  # (nchunks, 16, CHUNK)
        ol = ol.transpose(0, 2, 1).reshape(cap, 16)
        out[sel_c, 16:32] = ol[:npts]
    return out



# revision 12
# speedup vs baseline: 1.6422x; 1.0344x over previous
"""Trainium2 Bass kernel for nn_DecompGrid (factorized-grid embedding lookup).

Computation (per point, C=16 channels):
    out[n, 0:16]  = trilerp(grid3d, xyz) * bilerp(p0, (c1,c2)) * bilerp(p1, (c0,c2)) * bilerp(p2, (c0,c1))
    out[n, 16:32] = linelerp(line0, x[:, 3])

Strategy:
  - Host: compute cell indices + per-corner lerp weight PRODUCTS (cheap
    vectorized numpy), route points to the 8 cores by grid z-slab so the
    per-core grid table fits the dma_gather int16 index limit (<= 32768 rows),
    and build fp16 "dup-block" tables whose rows hold a full interpolation
    neighborhood in channel-major order (unit-stride inner dims for the DVE
    2x fp16 perf mode):
      grid:  (16ch, 8 corners) fp16 = 256B per row, 8*64*64 rows per core slab
      plane: (16ch, 4 corners) fp16 = 128B + 128B pad,  128*128 rows
  - Device (per chunk of 128*S points): load fp16 corner weights + wrapped
    int16 indices, 4x SWDGE dma_gather (one row per point per table, one table
    per SWDGE queue so the drains balance), fp16 DVE weighted combine, store
    (128, S, 16) fp16 spatial (host upcasts).
  - The line lerp runs on the otherwise-idle PE: stationary = (64, 16) line
    table, moving = host-built (64, CHUNK) sparse weight columns (1-w at i0,
    w at i0+1), PSUM -> fp16 via the scalar engine, stored as (16, CHUNK).

The hot loop is bound by SWDGE gather descriptor generation on the Pool
engine (~4 descriptors/point, ~2.8 ns each, serialized per instruction);
one full-chunk gather per table minimizes per-instruction overhead.
"""

import math
import numpy as np

import concourse.bacc as bacc
import concourse.bass as bass
import concourse.tile as tile
from concourse import mybir
from concourse import bass_utils

# ---------------- problem constants (hardcoded) ----------------
N = 1_000_000
C = 16
D = H = W = 128        # grid3d spatial dims
HP = WP = 256          # plane dims
LL = 64                # line length
NCORES = 8

S = 32                 # point-groups per partition per chunk
CHUNK = 128 * S        # points per chunk
NW = 24                # weight columns per point (8 grid + 12 plane + pad)

F32 = mybir.dt.float32
F16 = mybir.dt.float16
I16 = mybir.dt.int16


# ---------------- walrus / tile workarounds ----------------
_PATCHED = False


def _apply_patches():
    """This container's walrus rejects >1 sync-wait command on the Tile tail
    drain; split the waits into explicit wait_ge instructions."""
    global _PATCHED
    if _PATCHED:
        return
    _PATCHED = True
    import concourse.tile as tile_mod
    from concourse.tile import ScopedClock

    def _drain_and_barrier_split(self, tick_clock, wait_clock):
        drain_inst = self.nc.sync.drain()
        wait_clock.add_sem_waits(
            drain_inst.ins, ScopedClock({None: tick_clock.global_clock})
        )
        si = drain_inst.ins.sync_info
        if si is not None and len(si.on_wait) > 1:
            assert self.sems is not None
            by_name = {h.name: h for h in self.sems.allocated().values()}
            keep, spill = [], []
            for w in si.on_wait:
                h = by_name.get(w.ant_name)
                if h is None or len(keep) < 1:
                    keep.append(w)
                else:
                    spill.append((h, w.wait_value))
            si.on_wait = keep
            for h, v in spill:
                self.nc.sync.wait_ge(h, v)
        self.nc.all_engine_barrier()
        assert self.sems is not None
        popped = self.nc._tile_sem_poison_stack.pop()
        assert popped is self._sem_poison
        self.nc.clear_and_free_semaphores(list(self.sems.allocated().values()))
        self.nc.all_engine_barrier()

    tile_mod.TileContext._drain_and_barrier = _drain_and_barrier_split


# ---------------- device program ----------------

def build_program(nchunks: int, single_packet: bool = False):
    """Build + compile the SPMD bass program for `nchunks` chunks per core."""
    _apply_patches()
    nc = bacc.Bacc(
        "TRN2",
        num_devices=1,
        debug=False,
        target_bir_lowering=False,
        num_swdge_queues=4,
    )
    FS = S * 8   # idx cols per partition

    wts_d = nc.dram_tensor("wts", (nchunks, 128, S * NW), F16, kind="ExternalInput").ap()
    ig_d = nc.dram_tensor("idxg", (nchunks, 128, FS), I16, kind="ExternalInput").ap()
    ip0_d = nc.dram_tensor("idxp0", (nchunks, 128, FS), I16, kind="ExternalInput").ap()
    ip1_d = nc.dram_tensor("idxp1", (nchunks, 128, FS), I16, kind="ExternalInput").ap()
    ip2_d = nc.dram_tensor("idxp2", (nchunks, 128, FS), I16, kind="ExternalInput").ap()
    al_d = nc.dram_tensor("al", (nchunks, 64, CHUNK), F16, kind="ExternalInput").ap()
    gtab = nc.dram_tensor("gtab", (8 * 64 * 64, 128), F16, kind="ExternalInput").ap()
    p0tab = nc.dram_tensor("p0tab", (128 * 128, 128), F16, kind="ExternalInput").ap()
    p1tab = nc.dram_tensor("p1tab", (128 * 128, 128), F16, kind="ExternalInput").ap()
    p2tab = nc.dram_tensor("p2tab", (128 * 128, 128), F16, kind="ExternalInput").ap()
    ltab = nc.dram_tensor("ltab", (LL, C), F16, kind="ExternalInput").ap()
    out_d = nc.dram_tensor("out", (nchunks, 128, S * 16), F16, kind="ExternalOutput").ap()
    oln_d = nc.dram_tensor("oln", (nchunks, 16, CHUNK), F16, kind="ExternalOutput").ap()

    mul = mybir.AluOpType.mult
    add = mybir.AluOpType.add

    with tile.TileContext(nc) as tc:
        with tc.tile_pool(name="pconst", bufs=1) as pconst, \
             tc.tile_pool(name="pin", bufs=3) as pin, \
             tc.tile_pool(name="pval", bufs=4) as pval, \
             tc.tile_pool(name="ptmp", bufs=2) as ptmp, \
             tc.tile_pool(name="pps", bufs=4, space="PSUM") as pps, \
             tc.tile_pool(name="pout", bufs=2) as pout:
            lsb = pconst.tile([LL, C], F16, tag="lsb")
            nc.sync.dma_start(out=lsb[:], in_=ltab)

            for k in range(nchunks):
                # ---- loads ----
                wts = pin.tile([128, S, NW], F16, tag="wts")
                nc.sync.dma_start(out=wts[:], in_=wts_d[k].rearrange("p (s q) -> p s q", q=NW))
                ig = pin.tile([128, FS], I16, tag="ig")
                nc.sync.dma_start(out=ig[:], in_=ig_d[k])
                ip0 = pin.tile([128, FS], I16, tag="ip0")
                nc.sync.dma_start(out=ip0[:], in_=ip0_d[k])
                ip1 = pin.tile([128, FS], I16, tag="ip1")
                nc.sync.dma_start(out=ip1[:], in_=ip1_d[k])
                ip2 = pin.tile([128, FS], I16, tag="ip2")
                nc.sync.dma_start(out=ip2[:], in_=ip2_d[k])
                al = pin.tile([64, CHUNK], F16, tag="al")
                nc.sync.dma_start(out=al[:], in_=al_d[k])

                # ---- gathers (one full-chunk gather per table, one table per
                # SWDGE queue).  Full-chunk gathers halve the per-instruction
                # fixed cost + Pool context-wait stalls vs half-chunk splits
                # (measured 1.57ms -> 1.20ms). ----
                vg = pval.tile([128, S, 128], F16, tag="vg")
                vps = [pval.tile([128, S, 128], F16, tag=f"vp{t}", name=f"vp{t}")
                       for t in range(3)]
                tabs = ((vg, gtab, ig), (vps[0], p0tab, ip0),
                        (vps[1], p1tab, ip1), (vps[2], p2tab, ip2))
                for v, tab, ip in tabs:
                    nc.gpsimd.dma_gather(
                        v[:], tab, ip[:], CHUNK, CHUNK, 128,
                        queue_num=0, single_packet=single_packet)

                # ---- line lerp on PE: (64,16)^T @ (64,512) x 8 ----
                oln = pout.tile([16, CHUNK], F16, tag="oln")
                for j in range(CHUNK // 512):
                    ps = pps.tile([16, 512], F32, tag="ps")
                    nc.tensor.matmul(ps[:], lhsT=lsb[:], rhs=al[:, 512 * j:512 * (j + 1)])
                    nc.scalar.copy(out=oln[:, 512 * j:512 * (j + 1)], in_=ps[:])
                nc.sync.dma_start(out=oln_d[k], in_=oln[:])

                out_t = pout.tile([128, S, 16], F16, tag="out")
                out16 = out_t[:, :, 0:16]

                # ---- combine: grid (row = 16ch x 8 corners, fp16 2x mode) ----
                vg4 = vg[:].rearrange("p s (c k) -> p s c k", k=8)
                nc.vector.tensor_tensor(
                    out=vg4, in0=vg4,
                    in1=wts[:, :, 0:8].unsqueeze(2).broadcast_to([128, S, 16, 8]),
                    op=mul,
                )
                nc.vector.tensor_tensor(
                    out=vg4[:, :, :, 0:4], in0=vg4[:, :, :, 0:4], in1=vg4[:, :, :, 4:8], op=add)
                nc.vector.tensor_tensor(
                    out=vg4[:, :, :, 0:2], in0=vg4[:, :, :, 0:2], in1=vg4[:, :, :, 2:4], op=add)
                nc.vector.tensor_tensor(
                    out=out16, in0=vg4[:, :, :, 0], in1=vg4[:, :, :, 1], op=add)

                # ---- combine: planes (row = 16ch x 4 corners + pad) ----
                for t, v in enumerate(vps):
                    v4 = v[:, :, 0:64].rearrange("p s (c k) -> p s c k", k=4)
                    nc.vector.tensor_tensor(
                        out=v4, in0=v4,
                        in1=wts[:, :, 8 + 4 * t: 12 + 4 * t].unsqueeze(2)
                               .broadcast_to([128, S, 16, 4]),
                        op=mul,
                    )
                    nc.vector.tensor_tensor(
                        out=v4[:, :, :, 0:2], in0=v4[:, :, :, 0:2], in1=v4[:, :, :, 2:4],
                        op=add)
                    tsum = ptmp.tile([128, S, 16], F16, tag=f"ts{t}")
                    nc.vector.tensor_tensor(
                        out=tsum[:], in0=v4[:, :, :, 0], in1=v4[:, :, :, 1], op=add)
                    nc.vector.tensor_tensor(
                        out=out16, in0=out16, in1=tsum[:], op=mul)

                # ---- store ----
                nc.sync.dma_start(out=out_d[k], in_=out_t[:].rearrange("p s q -> p (s q)"))

    # Spread gathers across the 4 SWDGE queues (4 Q7 core pairs generate
    # descriptors in parallel). Tile assigned each Pool-DMA a DMASW{lane} sem
    # in scheduled order; a sem must always be fed by the same queue, so
    # derive queue_num = lane % 4.
    for bb in nc.m.functions[0].blocks:
        for inst in bb.instructions:
            if isinstance(inst, mybir.InstDMAGatherAnt):
                si = inst.sync_info
                for u in (si.on_update if si else []):
                    if u.ant_name.startswith("DMASW"):
                        lane = int(u.ant_name[5:].split("_")[0])
                        inst.queue_num = lane % 4
                        break
    nc.compile()
    return nc


_PROGRAM_CACHE = {}


def _get_program(nchunks: int):
    if nchunks not in _PROGRAM_CACHE:
        _PROGRAM_CACHE[nchunks] = build_program(nchunks)
    return _PROGRAM_CACHE[nchunks]


# ---------------- host-side preparation ----------------

def _split_idx_host(p, lo, hi):
    """Clamped floor + weight, matching the reference within [lo, hi+1]."""
    i0 = np.clip(np.floor(p), lo, hi).astype(np.int32)
    w = (p - i0.astype(np.float32)).astype(np.float32)
    return i0, w


def _build_tables(grid3d, plane0, plane1, plane2, line0):
    gT = np.ascontiguousarray(grid3d.transpose(1, 2, 3, 0)).astype(np.float16)  # (D,H,W,C)
    # per-core z-slab dup-block tables: core c owns z-origins 63+8c .. 63+8c+7
    # row layout: (16 ch, 8 corners) with corner = 4dz+2dy+dx
    gtabs = []
    for c in range(NCORES):
        z0 = 63 + 8 * c
        blk = np.empty((8, 64, 64, C, 2, 2, 2), np.float16)
        for dz in range(2):
            for dy in range(2):
                for dx in range(2):
                    blk[:, :, :, :, dz, dy, dx] = gT[
                        z0 + dz:z0 + dz + 8, 63 + dy:127 + dy, 63 + dx:127 + dx, :]
        gtabs.append(blk.reshape(8 * 64 * 64, 128))

    # plane rows: elems 0:64 = (16 ch, 4 corners) with corner = 2dy+dx; 64:128 pad
    ptabs = []
    for plane in (plane0, plane1, plane2):
        pT = np.ascontiguousarray(plane.transpose(1, 2, 0)).astype(np.float16)  # (H,W,C)
        blk = np.zeros((128, 128, 128), np.float16)
        core = blk[:, :, 0:64].reshape(128, 128, C, 2, 2)
        for dy in range(2):
            for dx in range(2):
                core[:, :, :, dy, dx] = pT[127 + dy:255 + dy, 127 + dx:255 + dx, :]
        ptabs.append(blk.reshape(128 * 128, 128))

    ltab = np.ascontiguousarray(line0.T).astype(np.float16)  # (L, C)
    return gtabs, ptabs, ltab


def _wrap_idx(idx_sorted, nchunks):
    """(cap,) int -> (nchunks, 128, 8S) int16 wrapped dma_gather layout."""
    a = idx_sorted.astype(np.int16).reshape(nchunks, S, 8, 16)
    a = a.transpose(0, 3, 1, 2).reshape(nchunks, 16, 8 * S)
    return np.ascontiguousarray(np.tile(a, (1, 8, 1)))


def _corner_weights(wgh, wph):
    """(npts, NW) f16: per-corner weight products in gathered-row order."""
    npts = wgh.shape[0]
    wts = np.zeros((npts, NW), np.float16)
    one = np.float32(1.0)
    wx, wy, wz = wgh[:, 0], wgh[:, 1], wgh[:, 2]
    zs = (one - wz, wz)
    ys = (one - wy, wy)
    xs = (one - wx, wx)
    for dz in range(2):
        for dy in range(2):
            zy = zs[dz] * ys[dy]
            for dx in range(2):
                wts[:, 4 * dz + 2 * dy + dx] = zy * xs[dx]
    w0, w1, w2 = wph[:, 0], wph[:, 1], wph[:, 2]
    for t, (cy, cx) in enumerate(((w2, w1), (w2, w0), (w1, w0))):
        cys = (one - cy, cy)
        cxs = (one - cx, cx)
        for dy in range(2):
            for dx in range(2):
                wts[:, 8 + 4 * t + 2 * dy + dx] = cys[dy] * cxs[dx]
    return wts


def kernel(x, grid3d, plane0, plane1, plane2, line0):
    x = np.asarray(x, np.float32)
    grid3d = np.asarray(grid3d, np.float32)
    plane0 = np.asarray(plane0, np.float32)
    plane1 = np.asarray(plane1, np.float32)
    plane2 = np.asarray(plane2, np.float32)
    line0 = np.asarray(line0, np.float32)

    npts_total = x.shape[0]
    half = np.float32(0.5)
    one = np.float32(1.0)

    # coordinates in the reference's f32 arithmetic order
    pg = ((x[:, 0:3] + one) * half) * np.float32(D - 1)   # grid:  coords 0,1,2
    pp = ((x[:, 0:3] + one) * half) * np.float32(HP - 1)  # plane coords
    pl = x[:, 3] * np.float32(LL - 1)

    i0g, wgh = _split_idx_host(pg, 63, 126)
    i0p, wph = _split_idx_host(pp, 127, 254)
    i0l, wlh = _split_idx_host(pl, 0, 62)

    # z-slab routing (grid z = coord 2). Points stay in arrival order within
    # a slab on purpose: cell-sorting them concentrates the 16 SDMA engines'
    # concurrent gather reads onto the same HBM bank region and serializes
    # the drain (measured 1.7x slower); random order spreads banks/channels.
    slab = (i0g[:, 2] - 63) >> 3
    order = np.argsort(slab, kind="stable")
    counts = np.bincount(slab, minlength=NCORES)
    cap_pts = int(counts.max())
    nchunks = max(1, math.ceil(cap_pts / CHUNK))
    cap = nchunks * CHUNK

    # per-point table indices (slab-local grid)
    idx_g = ((i0g[:, 2] - 63 - 8 * slab) * 64 + (i0g[:, 1] - 63)) * 64 + (i0g[:, 0] - 63)
    idx_p0 = (i0p[:, 2] - 127) * 128 + (i0p[:, 1] - 127)
    idx_p1 = (i0p[:, 2] - 127) * 128 + (i0p[:, 0] - 127)
    idx_p2 = (i0p[:, 1] - 127) * 128 + (i0p[:, 0] - 127)

    wts = _corner_weights(wgh, wph)
    wl16 = wlh.astype(np.float16)
    wl16c = (np.float32(1.0) - wlh).astype(np.float16)

    gtabs, ptabs, ltab = _build_tables(grid3d, plane0, plane1, plane2, line0)

    offs = np.zeros(NCORES + 1, np.int64)
    offs[1:] = np.cumsum(counts)

    in_maps = []
    for c in range(NCORES):
        sel = order[offs[c]:offs[c + 1]]
        npts = sel.shape[0]
        pad = cap - npts
        if pad:
            sel = np.concatenate([sel, np.repeat(sel[:1] if npts else [0], pad)])

        wtsc = wts[sel].reshape(nchunks, S, 128, NW).transpose(0, 2, 1, 3)
        wtsc = np.ascontiguousarray(wtsc.reshape(nchunks, 128, S * NW))
        alc = np.zeros((cap, LL), np.float16)
        ar = np.arange(cap)
        i0s = i0l[sel]
        alc[ar, i0s] = wl16c[sel]
        alc[ar, i0s + 1] = wl16[sel]
        alc = np.ascontiguousarray(alc.reshape(nchunks, CHUNK, LL).transpose(0, 2, 1))
        in_maps.append({
            "wts": wtsc,
            "idxg": _wrap_idx(idx_g[sel], nchunks),
            "idxp0": _wrap_idx(idx_p0[sel], nchunks),
            "idxp1": _wrap_idx(idx_p1[sel], nchunks),
            "idxp2": _wrap_idx(idx_p2[sel], nchunks),
            "al": alc,
            "gtab": gtabs[c],
            "p0tab": ptabs[0],
            "p1tab": ptabs[1],
            "p2tab": ptabs[2],
            "ltab": ltab,
        })

    nc = _get_program(nchunks)
    res = bass_utils.run_bass_kernel_spmd(nc, in_maps, core_ids=list(range(NCORES)))
    kernel.last_results = res

    out = np.empty((npts_total, 32), np.float32)
    for c in range(NCORES):
        sel_c = order[offs[c]:offs[c + 1]]
        npts = int(counts[c])
        o = res.results[c]["out"].astype(np.float32).reshape(nchunks, 128, S, 16)
        o = o.transpose(0, 2, 1, 3).reshape(cap, 16)
        out[sel_c, 0:16] = o[:npts]
        ol = res.results[c]["oln"].astype(np.float32)  # (nchunks, 16, CHUNK)
        ol = ol.transpose(0, 2, 1).reshape(cap, 16)
        out[sel_c, 16:32] = ol[:npts]
    return out

